# revision 10
# baseline (speedup 1.0000x reference)
"""Block-local self-attention (BLOCK=128, 3-block sliding window + global token 0)
for Trainium2, sharded over 8 NeuronCores by (batch*head).

Full shapes: q/k/v (2, 16, 4096, 64) fp32, mask (2, 1, 1, 4096) fp32 (zeros).
Core c handles 4 consecutive (n*16+h) heads, as 2 "head pairs".

Device kernel strategy (per head):
  - Q^T / K^T built as (d, t) bf16 tiles via gpsimd cast + xbar DMA transpose,
    two heads sharing the 128-partition dim (head A rows 0-63, head B rows 64-127).
  - Scores computed in S^T (key-partition, query-free) layout: per 512-query
    window, 5-6 matmul "pieces" (one per key block j covering its <=3 query
    blocks) packed into one (128, 1536) PSUM region.
  - exp on ScalarE (scale=1/8 folded into the activation affine) -> P^T bf16.
  - PV in ctx^T layout: ctx~ (65, 512) PSUM accumulates Vt_j^T @ P^T_j over
    pieces; row 64 is the softmax denominator via a ones-column in Vt.
  - Global token 0 ("global slot"): every query also attends token 0. exp of
    those scores (tiny: t per head) is precomputed on host and passed as the
    bf16 input `pg`; a rank-1 matmul [V0|1] x pg adds it to ctx~ and sums.
    pg is zeroed for query blocks 0,1 where token 0 is already inside the
    local window (reference masks the in-window slot and adds a global slot
    with identical score -> net effect: unmasked in-window token 0).
  - ctx~ -> SBUF bf16 -> PE transpose to (q, d) -> DVE reciprocal+multiply
    normalize -> fp32 out staging -> DMA.
Query token 0 (which attends the full sequence) is one row per head; it is
computed on host and patched into the output.
"""

import itertools
import math

import numpy as np
import ml_dtypes

N_, H, T, D = 2, 16, 4096, 64
B = 128
NB = T // B            # 32 key/query blocks
HPC = 4                # heads per core
NCORES = 8
WQ = 512               # queries per window
NWIN = T // WQ         # 8 windows per head
SCALE = 1.0 / math.sqrt(D)
BANK = 512             # fp32 elements per PSUM bank (per partition)


def _window_pieces(w):
    """Pieces for window w: (j, qb_lo, qb_hi, N) with q blocks in window units."""
    qb0, qb1 = 4 * w, 4 * w + 3
    out = []
    for j in range(max(0, qb0 - 1), min(NB - 1, qb1 + 1) + 1):
        qlo = max(qb0, j - 1)
        qhi = min(qb1, j + 1)
        out.append((j, qlo, qhi, (qhi - qlo + 1) * B))
    return out


def _pack_offsets(sizes):
    """Pack piece sizes contiguously from 0 s.t. no piece crosses a 512-elem
    PSUM bank boundary. Returns list of offsets (same order as sizes)."""
    n = len(sizes)
    for perm in itertools.permutations(range(n)):
        off = 0
        offs = [0] * n
        ok = True
        for i in perm:
            sz = sizes[i]
            if off // BANK != (off + sz - 1) // BANK:
                ok = False
                break
            offs[i] = off
            off += sz
        if ok:
            return offs
    raise ValueError(f"cannot pack {sizes}")


_NC_CACHE = {}


def _build_nc():
    if "nc" in _NC_CACHE:
        return _NC_CACHE["nc"]

    import concourse.bacc as bacc
    import concourse.bass as bass
    import concourse.mybir as mybir
    import concourse.tile as tile
    from concourse.masks import make_identity

    dt = mybir.dt
    F32, BF16 = dt.float32, dt.bfloat16

    nc = bacc.Bacc("TRN2", target_bir_lowering=False, debug=False)
    q_d = nc.dram_tensor("q", [HPC, T, D], F32, kind="ExternalInput")
    k_d = nc.dram_tensor("k", [HPC, T, D], F32, kind="ExternalInput")
    v_d = nc.dram_tensor("v", [HPC, T, D], F32, kind="ExternalInput")
    pg_d = nc.dram_tensor("pg", [HPC, T], BF16, kind="ExternalInput")
    o_d = nc.dram_tensor("o", [HPC, T, D], F32, kind="ExternalOutput")

    with tile.TileContext(nc) as tc:
        with (
            tc.tile_pool(name="singles", bufs=1) as singles,
            tc.tile_pool(name="xsrcp", bufs=1) as xsrcp,
            tc.tile_pool(name="qktp", bufs=1) as qktp,
            tc.tile_pool(name="vtp", bufs=4) as vtp,
            tc.tile_pool(name="pp", bufs=3) as pp,
            tc.tile_pool(name="ccp", bufs=2) as ccp,
            tc.tile_pool(name="rtp", bufs=2) as rtp,
            tc.tile_pool(name="outp", bufs=1) as outp,
            tc.tile_pool(name="spsum", bufs=2, space="PSUM") as spsum,
            tc.tile_pool(name="cpsum", bufs=1, space="PSUM") as cpsum,
            tc.tile_pool(name="tpsum", bufs=1, space="PSUM") as tpsum,
        ):
            identity = singles.tile([128, 128], BF16)
            make_identity(nc, identity[:, :])

            pgt = []
            for h in range(HPC):
                t_ = singles.tile([1, T], BF16, tag=f"pg{h}")
                nc.sync.dma_start(out=t_[:, :], in_=pg_d.ap()[h : h + 1, :])
                pgt.append(t_)

            # Prep. Q/K: SWDGE cast loads (fp32->bf16) into pair tiles with
            # heads side by side on the 128-col dim, then per-8-block xbar
            # transposes on the sync HWDGE ring. Tile tracks dependencies at
            # whole-tile granularity and coalesces sem waits by FIFO
            # dominance, so each xsrc tile must receive ALL its writes
            # before its first reader and no tile may be written after a
            # transpose has read it -- hence separate tiles per (pair,
            # tensor, chunk-group), loads all emitted before all
            # transposes, both in consume order.
            CHK = 8
            qt_pair, kt_pair, vt = [None, None], [None, None], [None] * HPC

            def load_xsrc(src_d, pair, lo, hi, xsrc):
                hA, hB = 2 * pair, 2 * pair + 1
                for hi_, h in enumerate((hA, hB)):
                    nc.gpsimd.dma_start(
                        out=xsrc[:, 0 : hi - lo, hi_ * 64 : hi_ * 64 + 64],
                        in_=src_d.ap()[h, lo * B : hi * B, :].rearrange(
                            "(c p) d -> p c d", p=B
                        ),
                    )

            def load_v(h):
                vt_h = vtp.tile([128, NB, D + 1], BF16, tag="vt")
                nc.gpsimd.dma_start(
                    out=vt_h[:, :, 0:D],
                    in_=v_d.ap()[h].rearrange("(c p) d -> p c d", p=B),
                )
                nc.gpsimd.memset(vt_h[:, :, D : D + 1], 1.0)
                vt[h] = vt_h

            xk0a = xsrcp.tile([128, CHK, 128], BF16, tag="xk0a")
            xq0a = xsrcp.tile([128, CHK, 128], BF16, tag="xq0a")
            xk0b = xsrcp.tile([128, NB - CHK, 128], BF16, tag="xk0b")
            xq0b = xsrcp.tile([128, NB - CHK, 128], BF16, tag="xq0b")
            xk1 = xsrcp.tile([128, NB, 128], BF16, tag="xk1")
            xq1 = xsrcp.tile([128, NB, 128], BF16, tag="xq1")
            # kt/qt as per-chunk tiles so scores only wait on the chunk's
            # own transpose (whole-tile dep granularity). Window query
            # ranges are 4-block aligned so no matmul operand crosses an
            # 8-block chunk boundary.
            NCH = NB // CHK
            for pair in range(2):
                ktc, qtc = [], []
                for c in range(NCH):
                    kt_c = qktp.tile([128, CHK, 128], BF16, tag=f"kt{pair}{c}")
                    qt_c = qktp.tile([128, CHK, 128], BF16, tag=f"qt{pair}{c}")
                    ktc.append(kt_c)
                    qtc.append(qt_c)
                kt_pair[pair], qt_pair[pair] = ktc, qtc

            # gpsimd (SWDGE) queue order == data-need order.
            load_xsrc(k_d, 0, 0, CHK, xk0a)
            load_xsrc(q_d, 0, 0, CHK, xq0a)
            load_v(0)
            load_xsrc(k_d, 0, CHK, NB, xk0b)
            load_xsrc(q_d, 0, CHK, NB, xq0b)
            load_v(1)
            load_xsrc(k_d, 1, 0, NB, xk1)
            load_xsrc(q_d, 1, 0, NB, xq1)
            load_v(2)
            load_v(3)

            # sync queue: all 16 xbar transposes up front, in consume order.
            def xpose(tt_c, xsrc, src_c0):
                nc.sync.dma_start_transpose(
                    tt_c[:, :, :], xsrc[:, src_c0 : src_c0 + CHK, :]
                )

            xpose(kt_pair[0][0], xk0a, 0)
            xpose(qt_pair[0][0], xq0a, 0)
            for c in range(1, NCH):
                xpose(kt_pair[0][c], xk0b, (c - 1) * CHK)
            for c in range(1, NCH):
                xpose(qt_pair[0][c], xq0b, (c - 1) * CHK)
            for c in range(NCH):
                xpose(kt_pair[1][c], xk1, c * CHK)
            for c in range(NCH):
                xpose(qt_pair[1][c], xq1, c * CHK)

            # Compute, software-pipelined across a flat (head, window) job
            # list with lag-3: at step `it` we emit scores+exp for job it,
            # PV+psum-copy for job it-2, transpose+normalize for job it-3.
            # Every cross-engine dependency then has >=1 full iteration of
            # slack, so PE semaphore waits are pre-satisfied and the PE
            # stream stays contiguous (HAM un-throttles to 2.4 GHz only
            # under long wait-free bursts). Jobs run pair-0 heads first
            # (windows interleaved across the head pair) so compute can
            # start as soon as the first Q/K chunk is transposed.
            LAG_PV, LAG_T = 2, 3
            jobs = [(h, w) for w in range(NWIN) for h in (0, 1)] + [
                (h, w) for w in range(NWIN) for h in (2, 3)
            ]
            # output staged in half-head tiles so the first-half store
            # doesn't WAR-serialize against later windows' normalize.
            outstage = []
            for h in range(HPC):
                out_h0 = outp.tile([128, NB // 2, D], F32, tag=f"out{h}a")
                out_h1 = outp.tile([128, NB // 2, D], F32, tag=f"out{h}b")
                outstage.append((out_h0, out_h1))
            state = {}
            for it in range(len(jobs) + LAG_T):
                if it < len(jobs):
                    h, w = jobs[it]
                    pair, dlo = h // 2, (h % 2) * 64
                    ktc, qtc = kt_pair[pair], qt_pair[pair]
                    cw = w // 2  # q chunk of this window
                    pieces = _window_pieces(w)
                    offs = _pack_offsets([p[3] for p in pieces])
                    tot = sum(p[3] for p in pieces)
                    sc = spsum.tile([128, 3 * BANK], F32, tag="sc")
                    for (j, qlo, qhi, n), off in zip(pieces, offs):
                        nc.tensor.matmul(
                            out=sc[:, off : off + n],
                            lhsT=ktc[j // CHK][dlo : dlo + 64, j % CHK, :],
                            rhs=qtc[cw][
                                dlo : dlo + 64,
                                qlo - cw * CHK : qhi + 1 - cw * CHK,
                                :,
                            ],
                            start=True,
                            stop=True,
                        )
                    P = pp.tile([128, 3 * BANK], BF16, tag="p")
                    nc.scalar.activation(
                        out=P[:, 0:tot],
                        in_=sc[:, 0:tot],
                        func=mybir.ActivationFunctionType.Exp,
                        scale=SCALE,
                    )
                    state[it] = (h, w, pieces, offs, P)
                if 0 <= it - LAG_PV < len(jobs):
                    h, w, pieces, offs, P = state[it - LAG_PV]
                    ctx = cpsum.tile([D + 1, WQ], F32, tag="ctx")
                    # rank-1 global-token term first: it covers the full
                    # (65, 512) region, so the accumulation group starts with
                    # every element freshly written (the simulator requires
                    # uniform fresh-vs-accumulate per instruction).
                    nc.tensor.matmul(
                        out=ctx[:, :],
                        lhsT=vt[h][0:1, 0, :],
                        rhs=pgt[h][:, w * WQ : (w + 1) * WQ],
                        start=True,
                        stop=False,
                    )
                    for i, ((j, qlo, qhi, n), off) in enumerate(zip(pieces, offs)):
                        nc.tensor.matmul(
                            out=ctx[:, (qlo - 4 * w) * B : (qhi + 1 - 4 * w) * B],
                            lhsT=vt[h][:, j, :],
                            rhs=P[:, off : off + n],
                            start=False,
                            stop=(i == len(pieces) - 1),
                        )
                    ctxC = ccp.tile([D + 1, WQ], BF16, tag="cc")
                    nc.vector.tensor_copy(out=ctxC[:, :], in_=ctx[:, :])
                    state[it - LAG_PV] = (h, w, ctxC)
                if 0 <= it - LAG_T < len(jobs):
                    h, w, ctxC = state.pop(it - LAG_T)
                    ctxT = tpsum.tile([128, 4, D + 2], BF16, tag="ct")
                    for c in range(4):
                        nc.tensor.transpose(
                            ctxT[:, c, 0 : D + 1],
                            ctxC[:, c * B : (c + 1) * B],
                            identity[0 : D + 1, 0 : D + 1],
                        )
                    rt = rtp.tile([128, 4], F32, tag="rt")
                    nc.vector.reciprocal(out=rt[:, :], in_=ctxT[:, :, D : D + 1])
                    half, hb = w // (NWIN // 2), (4 * w) % (NB // 2)
                    nc.vector.tensor_mul(
                        out=outstage[h][half][:, hb : hb + 4, :],
                        in0=ctxT[:, :, 0:D],
                        in1=rt[:, :].broadcast_to([128, 4, D]),
                    )
                    if w == NWIN // 2 - 1 or w == NWIN - 1:
                        nc.sync.dma_start(
                            out=o_d.ap()[
                                h, half * (T // 2) : (half + 1) * (T // 2), :
                            ].rearrange("(c p) d -> p c d", p=B),
                            in_=outstage[h][half][:, :, :],
                        )

    nc.compile()
    _NC_CACHE["nc"] = nc
    return nc


def _host_globals(query, key, value):
    """Host-side tiny pieces: pg = exp(scale * K0 . Q) (zeroed for the first
    two query blocks), and o0 = full-sequence attention output for query 0
    (token 0 masked out, as the reference does via attention_mask[..., 0])."""
    q = np.asarray(query, np.float32)
    k = np.asarray(key, np.float32)
    v = np.asarray(value, np.float32)
    k0 = k[:, :, 0, :]  # (n, h, d)
    sg = np.einsum("nhd,nhtd->nht", k0, q) * SCALE
    pg = np.exp(sg)
    pg[:, :, : 2 * B] = 0.0

    q0 = q[:, :, 0, :]  # (n, h, d)
    s0 = np.einsum("nhd,nhtd->nht", q0, k) * SCALE
    s0[:, :, 0] = -np.inf
    s0 -= s0.max(axis=-1, keepdims=True)
    p0 = np.exp(s0)
    p0 /= p0.sum(axis=-1, keepdims=True)
    o0 = np.einsum("nht,nhtd->nhd", p0, v)
    return pg, o0


def kernel(query_layer, key_layer, value_layer, attention_mask):
    from concourse.bass_utils import run_bass_kernel_spmd

    n, h, t, d = query_layer.shape
    assert (n, h, t, d) == (N_, H, T, D)

    q = np.ascontiguousarray(np.asarray(query_layer, np.float32))
    k = np.ascontiguousarray(np.asarray(key_layer, np.float32))
    v = np.ascontiguousarray(np.asarray(value_layer, np.float32))
    pg, o0 = _host_globals(q, k, v)

    qf = q.reshape(n * h, T, D)
    kf = k.reshape(n * h, T, D)
    vf = v.reshape(n * h, T, D)
    pgf = pg.reshape(n * h, T).astype(ml_dtypes.bfloat16)

    in_maps = []
    for c in range(NCORES):
        s = slice(HPC * c, HPC * (c + 1))
        in_maps.append(
            {
                "q": np.ascontiguousarray(qf[s]),
                "k": np.ascontiguousarray(kf[s]),
                "v": np.ascontiguousarray(vf[s]),
                "pg": np.ascontiguousarray(pgf[s]),
            }
        )

    nc = _build_nc()
    res = run_bass_kernel_spmd(nc, in_maps, core_ids=list(range(NCORES)))
    _NC_CACHE["last_result"] = res
    out = np.concatenate([r["o"] for r in res.results], axis=0)  # (n*h, T, D)
    out = out.reshape(n, h, T, D).copy()
    out[:, :, 0, :] = o0
    return out



# revision 18
# speedup vs baseline: 1.0877x; 1.0877x over previous
"""Block-local self-attention (BLOCK=128, 3-block sliding window + global token 0)
for Trainium2, sharded over 8 NeuronCores by (batch*head).

Full shapes: q/k/v (2, 16, 4096, 64) fp32, mask (2, 1, 1, 4096) fp32 (zeros).
Core c handles 4 consecutive (n*16+h) heads, as 2 "head pairs".

Device kernel strategy (per head):
  - Q^T / K^T built as (d, t) bf16 tiles via gpsimd cast + xbar DMA transpose,
    two heads sharing the 128-partition dim (head A rows 0-63, head B rows 64-127).
  - Scores computed in S^T (key-partition, query-free) layout: per 512-query
    window, 5-6 matmul "pieces" (one per key block j covering its <=3 query
    blocks) packed into one (128, 1536) PSUM region.
  - exp on ScalarE (scale=1/8 folded into the activation affine) -> P^T bf16.
  - PV in ctx^T layout: ctx~ (65, 512) PSUM accumulates Vt_j^T @ P^T_j over
    pieces; row 64 is the softmax denominator via a ones-column in Vt.
  - Global token 0 ("global slot"): every query also attends token 0. exp of
    those scores (tiny: t per head) is precomputed on host and passed as the
    bf16 input `pg`; a rank-1 matmul [V0|1] x pg adds it to ctx~ and sums.
    pg is zeroed for query blocks 0,1 where token 0 is already inside the
    local window (reference masks the in-window slot and adds a global slot
    with identical score -> net effect: unmasked in-window token 0).
  - ctx~ -> SBUF bf16 -> PE transpose to (q, d) -> DVE reciprocal+multiply
    normalize -> fp32 out staging -> DMA.
Query token 0 (which attends the full sequence) is one row per head; it is
computed on host and patched into the output.
"""

import itertools
import math

import numpy as np
import ml_dtypes

N_, H, T, D = 2, 16, 4096, 64
B = 128
NB = T // B            # 32 key/query blocks
HPC = 4                # heads per core
NCORES = 8
WQ = 512               # queries per window
NWIN = T // WQ         # 8 windows per head
SCALE = 1.0 / math.sqrt(D)
BANK = 512             # fp32 elements per PSUM bank (per partition)


def _window_pieces(w):
    """Pieces for window w: (j, qb_lo, qb_hi, N) with q blocks in window units."""
    qb0, qb1 = 4 * w, 4 * w + 3
    out = []
    for j in range(max(0, qb0 - 1), min(NB - 1, qb1 + 1) + 1):
        qlo = max(qb0, j - 1)
        qhi = min(qb1, j + 1)
        out.append((j, qlo, qhi, (qhi - qlo + 1) * B))
    return out


def _pack_offsets(sizes):
    """Pack piece sizes contiguously from 0 s.t. no piece crosses a 512-elem
    PSUM bank boundary. Returns list of offsets (same order as sizes)."""
    n = len(sizes)
    for perm in itertools.permutations(range(n)):
        off = 0
        offs = [0] * n
        ok = True
        for i in perm:
            sz = sizes[i]
            if off // BANK != (off + sz - 1) // BANK:
                ok = False
                break
            offs[i] = off
            off += sz
        if ok:
            return offs
    raise ValueError(f"cannot pack {sizes}")


_NC_CACHE = {}


def _build_nc():
    if "nc" in _NC_CACHE:
        return _NC_CACHE["nc"]

    import concourse.bacc as bacc
    import concourse.bass as bass
    import concourse.mybir as mybir
    import concourse.tile as tile
    from concourse.masks import make_identity

    dt = mybir.dt
    F32, BF16 = dt.float32, dt.bfloat16

    nc = bacc.Bacc("TRN2", target_bir_lowering=False, debug=False)
    # host-marshalled layouts: K/Q pair-major with the head pair interleaved
    # inside each row (so one DMA covers both heads with a 3-dim AP); V
    # t-major with all 4 heads + the ones column inside each row.
    q_d = nc.dram_tensor("q", [2, T, 2, D], F32, kind="ExternalInput")
    k_d = nc.dram_tensor("k", [2, T, 2, D], F32, kind="ExternalInput")
    v_d = nc.dram_tensor("v", [T, HPC, D + 1], F32, kind="ExternalInput")
    pg_d = nc.dram_tensor("pg", [1, HPC * T], BF16, kind="ExternalInput")
    o_d = nc.dram_tensor("o", [HPC, T, D], F32, kind="ExternalOutput")

    with tile.TileContext(nc) as tc:
        with (
            tc.tile_pool(name="singles", bufs=1) as singles,
            tc.tile_pool(name="xsrcp", bufs=1) as xsrcp,
            tc.tile_pool(name="qktp", bufs=1) as qktp,
            tc.tile_pool(name="vtp", bufs=1) as vtp,
            tc.tile_pool(name="pp", bufs=3) as pp,
            tc.tile_pool(name="ccp", bufs=2) as ccp,
            tc.tile_pool(name="rtp", bufs=2) as rtp,
            tc.tile_pool(name="outp", bufs=1) as outp,
            tc.tile_pool(name="spsum", bufs=2, space="PSUM") as spsum,
            tc.tile_pool(name="cpsum", bufs=1, space="PSUM") as cpsum,
            tc.tile_pool(name="tpsum", bufs=1, space="PSUM") as tpsum,
        ):
            identity = singles.tile([128, 128], BF16)
            make_identity(nc, identity[:, :])

            # DMA budget: the Tile scheduler has 8 SWDGE + 8 HWDGE
            # completion-sem slots, assigned round-robin. A slot can only be
            # reused once ALL waiters of its previous user have passed, so
            # >8 DMAs per class chains later DMAs behind unrelated consumers
            # (this serialized the whole load phase in earlier revisions).
            # SWDGE (gpsimd) therefore carries EXACTLY 8 loads (pg, V-all,
            # 6 Q/K pair loads) and never recycles a slot; HWDGE (sync)
            # carries the 8 xbar transposes in slots 1-8 and lets the 5
            # output stores reuse slots of transposes whose waiters are
            # long gone by store time.
            CHK = 16
            NCH = NB // CHK

            # pg: one flat [1, 4*T] row, single SWDGE load, slot 1.
            pgt = singles.tile([1, HPC * T], BF16, tag="pg")
            nc.gpsimd.dma_start(out=pgt[:, :], in_=pg_d.ap()[:, :])

            # Q/K pair tiles (head A cols 0-63, head B 64-127).
            xk0a = xsrcp.tile([128, CHK, 128], BF16, tag="xk0a")
            xq0a = xsrcp.tile([128, CHK, 128], BF16, tag="xq0a")
            xk0b = xsrcp.tile([128, CHK, 128], BF16, tag="xk0b")
            xq0b = xsrcp.tile([128, CHK, 128], BF16, tag="xq0b")
            xk1 = xsrcp.tile([128, NB, 128], BF16, tag="xk1")
            xq1 = xsrcp.tile([128, NB, 128], BF16, tag="xq1")

            def load_pair(src_d, pair, lo, hi, xsrc):
                nc.gpsimd.dma_start(
                    out=xsrc[:, 0 : hi - lo, :],
                    in_=src_d.ap()[pair, lo * B : hi * B, :, :].rearrange(
                        "(c p) h d -> p c (h d)", p=B
                    ),
                )

            # V: all four heads in ONE tile / ONE cast load; col D is the
            # ones-column (softmax denominator via PV row 64), packed into
            # the dram tensor by the host so no on-device memset is needed
            # (a memset after the load would block the gpsimd queue on the
            # V DMA's completion).
            vt = vtp.tile([128, NB, HPC * (D + 1)], BF16, tag="vt")

            # SWDGE queue order == data-need order (8 ops, 8 slots).
            load_pair(k_d, 0, 0, CHK, xk0a)
            load_pair(q_d, 0, 0, CHK, xq0a)
            nc.gpsimd.dma_start(
                out=vt[:, :, :],
                in_=v_d.ap()[:, :, :].rearrange("(c p) h e -> p c (h e)", p=B),
            )
            load_pair(k_d, 0, CHK, NB, xk0b)
            load_pair(q_d, 0, CHK, NB, xq0b)
            load_pair(k_d, 1, 0, NB, xk1)
            load_pair(q_d, 1, 0, NB, xq1)

            # kt/qt per-chunk tiles: consumers wait only on their own
            # chunk's transpose (whole-tile dep granularity).
            kt_pair, qt_pair = [None, None], [None, None]
            for pair in range(2):
                ktc, qtc = [], []
                for c in range(NCH):
                    kt_c = qktp.tile([128, CHK, 128], BF16, tag=f"kt{pair}{c}")
                    qt_c = qktp.tile([128, CHK, 128], BF16, tag=f"qt{pair}{c}")
                    ktc.append(kt_c)
                    qtc.append(qt_c)
                kt_pair[pair], qt_pair[pair] = ktc, qtc

            # sync queue: the 8 xbar transposes, in consume order.
            def xpose(tt_c, xsrc, src_c0):
                nc.sync.dma_start_transpose(
                    tt_c[:, :, :], xsrc[:, src_c0 : src_c0 + CHK, :]
                )

            xpose(kt_pair[0][0], xk0a, 0)
            xpose(qt_pair[0][0], xq0a, 0)
            xpose(kt_pair[0][1], xk0b, 0)
            xpose(qt_pair[0][1], xq0b, 0)
            xpose(kt_pair[1][0], xk1, 0)
            xpose(qt_pair[1][0], xq1, 0)
            xpose(kt_pair[1][1], xk1, CHK)
            xpose(qt_pair[1][1], xq1, CHK)

            # Compute, software-pipelined across a flat (head, window) job
            # list: iteration `it` emits ctx-transpose+normalize for job
            # it-3 FIRST, then scores+exp for job it, then PV+psum-copy for
            # job it-2. Emitting the transposes first makes the DVE finish
            # each iteration with the ctx copy (CAST), so the PE's
            # FIFO-coalesced waits on the DVE sem point at work that is
            # already done and the PE stream stays contiguous. Pair-0 heads
            # run first (windows interleaved across the head pair) so
            # compute starts as soon as the first Q/K chunk is transposed.
            LAG_PV, LAG_T = 2, 3
            jobs = [(h, w) for w in range(NWIN) for h in (0, 1)] + [
                (h, w) for w in range(NWIN) for h in (2, 3)
            ]
            out01 = outp.tile([128, NB, D], F32, tag="out0")
            out11 = outp.tile([128, NB, D], F32, tag="out1")
            out21 = outp.tile([128, NB, D], F32, tag="out2")
            out3a = outp.tile([128, NB // 2, D], F32, tag="out3a")
            out3b = outp.tile([128, NB // 2, D], F32, tag="out3b")
            outstage = {0: out01, 1: out11, 2: out21}
            state = {}
            for it in range(len(jobs) + LAG_T):
                if 0 <= it - LAG_T < len(jobs):
                    h, w, ctxC = state.pop(it - LAG_T)
                    ctxT = tpsum.tile([128, 4, D + 2], BF16, tag="ct")
                    for c in range(4):
                        nc.tensor.transpose(
                            ctxT[:, c, 0 : D + 1],
                            ctxC[:, c * B : (c + 1) * B],
                            identity[0 : D + 1, 0 : D + 1],
                        )
                    rt = rtp.tile([128, 4], F32, tag="rt")
                    nc.vector.reciprocal(out=rt[:, :], in_=ctxT[:, :, D : D + 1])
                    if h < 3:
                        dst = outstage[h][:, 4 * w : 4 * w + 4, :]
                    else:
                        half = w // (NWIN // 2)
                        ht = out3a if half == 0 else out3b
                        dst = ht[:, (4 * w) % (NB // 2) : (4 * w) % (NB // 2) + 4, :]
                    nc.vector.tensor_mul(
                        out=dst,
                        in0=ctxT[:, :, 0:D],
                        in1=rt[:, :].broadcast_to([128, 4, D]),
                    )
                    if h < 3 and w == NWIN - 1:
                        nc.sync.dma_start(
                            out=o_d.ap()[h].rearrange("(c p) d -> p c d", p=B),
                            in_=outstage[h][:, :, :],
                        )
                    elif h == 3 and (w == NWIN // 2 - 1 or w == NWIN - 1):
                        half = w // (NWIN // 2)
                        ht = out3a if half == 0 else out3b
                        nc.sync.dma_start(
                            out=o_d.ap()[
                                3, half * (T // 2) : (half + 1) * (T // 2), :
                            ].rearrange("(c p) d -> p c d", p=B),
                            in_=ht[:, :, :],
                        )
                if it < len(jobs):
                    h, w = jobs[it]
                    pair, dlo = h // 2, (h % 2) * 64
                    ktc, qtc = kt_pair[pair], qt_pair[pair]
                    cw = (4 * w) // CHK  # q chunk of this window
                    pieces = _window_pieces(w)
                    offs = _pack_offsets([p[3] for p in pieces])
                    tot = sum(p[3] for p in pieces)
                    sc = spsum.tile([128, 3 * BANK], F32, tag="sc")
                    for (j, qlo, qhi, n), off in zip(pieces, offs):
                        nc.tensor.matmul(
                            out=sc[:, off : off + n],
                            lhsT=ktc[j // CHK][dlo : dlo + 64, j % CHK, :],
                            rhs=qtc[cw][
                                dlo : dlo + 64,
                                qlo - cw * CHK : qhi + 1 - cw * CHK,
                                :,
                            ],
                            start=True,
                            stop=True,
                        )
                    P = pp.tile([128, 3 * BANK], BF16, tag="p")
                    nc.scalar.activation(
                        out=P[:, 0:tot],
                        in_=sc[:, 0:tot],
                        func=mybir.ActivationFunctionType.Exp,
                        scale=SCALE,
                    )
                    state[it] = (h, w, pieces, offs, P)
                if 0 <= it - LAG_PV < len(jobs):
                    h, w, pieces, offs, P = state[it - LAG_PV]
                    ctx = cpsum.tile([D + 1, WQ], F32, tag="ctx")
                    # rank-1 global-token term first: it covers the full
                    # (65, 512) region, so the accumulation group starts with
                    # every element freshly written (the simulator requires
                    # uniform fresh-vs-accumulate per instruction).
                    nc.tensor.matmul(
                        out=ctx[:, :],
                        lhsT=vt[0:1, 0, h * (D + 1) : (h + 1) * (D + 1)],
                        rhs=pgt[:, h * T + w * WQ : h * T + (w + 1) * WQ],
                        start=True,
                        stop=False,
                    )
                    for i, ((j, qlo, qhi, n), off) in enumerate(zip(pieces, offs)):
                        nc.tensor.matmul(
                            out=ctx[:, (qlo - 4 * w) * B : (qhi + 1 - 4 * w) * B],
                            lhsT=vt[:, j, h * (D + 1) : (h + 1) * (D + 1)],
                            rhs=P[:, off : off + n],
                            start=False,
                            stop=(i == len(pieces) - 1),
                        )
                    ctxC = ccp.tile([D + 1, WQ], BF16, tag="cc")
                    nc.vector.tensor_copy(out=ctxC[:, :], in_=ctx[:, :])
                    state[it - LAG_PV] = (h, w, ctxC)

    nc.compile()
    _NC_CACHE["nc"] = nc
    return nc



def _host_globals(query, key, value):
    """Host-side tiny pieces: pg = exp(scale * K0 . Q) (zeroed for the first
    two query blocks), and o0 = full-sequence attention output for query 0
    (token 0 masked out, as the reference does via attention_mask[..., 0])."""
    q = np.asarray(query, np.float32)
    k = np.asarray(key, np.float32)
    v = np.asarray(value, np.float32)
    k0 = k[:, :, 0, :]  # (n, h, d)
    sg = np.einsum("nhd,nhtd->nht", k0, q) * SCALE
    pg = np.exp(sg)
    pg[:, :, : 2 * B] = 0.0

    q0 = q[:, :, 0, :]  # (n, h, d)
    s0 = np.einsum("nhd,nhtd->nht", q0, k) * SCALE
    s0[:, :, 0] = -np.inf
    s0 -= s0.max(axis=-1, keepdims=True)
    p0 = np.exp(s0)
    p0 /= p0.sum(axis=-1, keepdims=True)
    o0 = np.einsum("nht,nhtd->nhd", p0, v)
    return pg, o0


def kernel(query_layer, key_layer, value_layer, attention_mask):
    from concourse.bass_utils import run_bass_kernel_spmd

    n, h, t, d = query_layer.shape
    assert (n, h, t, d) == (N_, H, T, D)

    q = np.ascontiguousarray(np.asarray(query_layer, np.float32))
    k = np.ascontiguousarray(np.asarray(key_layer, np.float32))
    v = np.ascontiguousarray(np.asarray(value_layer, np.float32))
    pg, o0 = _host_globals(q, k, v)

    # pair-major, head-interleaved K/Q: (n*h, T, D) -> per-core (2, T, 2, D);
    # V with ones column: per-core (T, 4, D+1).
    qf = q.reshape(n * h, T, D)
    kf = k.reshape(n * h, T, D)
    vf = np.concatenate(
        [v.reshape(n * h, T, D), np.ones((n * h, T, 1), np.float32)], axis=-1
    )
    pgf = pg.reshape(n * h, T).astype(ml_dtypes.bfloat16)

    in_maps = []
    for c in range(NCORES):
        s = slice(HPC * c, HPC * (c + 1))
        in_maps.append(
            {
                "q": np.ascontiguousarray(
                    qf[s].reshape(2, 2, T, D).transpose(0, 2, 1, 3)
                ),
                "k": np.ascontiguousarray(
                    kf[s].reshape(2, 2, T, D).transpose(0, 2, 1, 3)
                ),
                "v": np.ascontiguousarray(vf[s].transpose(1, 0, 2)),
                "pg": np.ascontiguousarray(pgf[s].reshape(1, HPC * T)),
            }
        )

    nc = _build_nc()
    res = run_bass_kernel_spmd(nc, in_maps, core_ids=list(range(NCORES)))
    _NC_CACHE["last_result"] = res
    out = np.concatenate([r["o"] for r in res.results], axis=0)  # (n*h, T, D)
    out = out.reshape(n, h, T, D).copy()
    out[:, :, 0, :] = o0
    return out

